# revision 12
# baseline (speedup 1.0000x reference)
"""DigitCaps dynamic-routing kernel for 8 Trainium2 NeuronCores.

Problem (hardcoded shapes): x [64,8,8,32,8] f32, W [2048,8,512] f32,
bias [32,16] f32 -> v [64,32,16] f32.  3 routing iterations.

Strategy: data-parallel over batch B (8 batches per core, W replicated).
Per core:
  - u_hat = einsum('bji,jik->bjk') built once on the tensor engine via
    block-diagonal lhsT packing (16 n's per matmul, K=128=16n*8i,
    M=128=16n*8b), converted to fp16 and kept *resident in SBUF* in
    layout A: UA[p=n%128, nt=n//128, b, cl]  (128 KB/partition).
  - each routing iteration:
      agreement: per (b,nt,cl-chunk) DMA-xbar-transpose a [128n,128cl]
        chunk of UA into [cl,n] and matmul against a block-diagonal
        Vbd[cl, 32] built from v -> psum[n, 32] accumulated over chunks.
      softmax over c on ACT(exp)+DVE.
      s: matmul lhsT=c[n,32] (fp16) rhs=UA[n,512] -> psum[32c', 512(c,l)]
        for 4 batches per PSUM bank; diagonal blocks extracted with a
        0/1 mask + strided reduce; squash on ACT/DVE.
  - v of the last iteration is written out in a [256,16] scratch layout
    and unscrambled on the host.
"""

import os
import sys

import numpy as np

if "/opt/trn_rl_repo" not in sys.path:
    sys.path.insert(0, "/opt/trn_rl_repo")

B, N, IL = 64, 2048, 8
C, L = 32, 16
CL = C * L  # 512
NCORES = 8
BL = B // NCORES  # 8 batches per core
NT = N // 128  # 16 n-tiles
EPS = 1e-7
R_ITERS = 3


def _build_program():
    import concourse.bacc as bacc
    import concourse.bass as bass
    import concourse.mybir as mybir
    import concourse.tile as tile
    from concourse.bass import ds

    f16 = mybir.dt.float16
    f32 = mybir.dt.float32
    AX = mybir.AxisListType.X
    Exp = mybir.ActivationFunctionType.Exp
    Sqrt = mybir.ActivationFunctionType.Sqrt
    Square = mybir.ActivationFunctionType.Square

    nc = bacc.Bacc()

    ubd_d = nc.dram_tensor("ubd", [128, 128, 128], f16, kind="ExternalInput")
    wst_d = nc.dram_tensor("wst", [128, 128, 512], f16, kind="ExternalInput")
    c0_d = nc.dram_tensor("c0", [128, 32], f16, kind="ExternalInput")
    msk_d = nc.dram_tensor("msk", [128, 512], f16, kind="ExternalInput")
    eall_d = nc.dram_tensor("eall", [128, 128], f16, kind="ExternalInput")
    bias4_d = nc.dram_tensor("bias4", [128, 16], f32, kind="ExternalInput")
    vout_d = nc.dram_tensor("vout", [256, 16], f32, kind="ExternalOutput")

    with tile.TileContext(nc) as tc:
        with tc.tile_pool(name="res", bufs=1) as rpool:
            C0 = rpool.tile([128, 32], f16, tag="c0")
            nc.sync.dma_start(C0[:], c0_d[:, :])
            MSK = rpool.tile([128, 512], f16, tag="msk")
            nc.sync.dma_start(MSK[:], msk_d[:, :])
            EALL = rpool.tile([128, 128], f16, tag="eall")
            nc.sync.dma_start(EALL[:], eall_d[:, :])
            BIAS4 = rpool.tile([128, 16], f32, tag="bias4")
            nc.sync.dma_start(BIAS4[:], bias4_d[:, :])

            UA = rpool.tile([128, NT, BL, CL], f16, tag="ua")
            LOG = rpool.tile([128, BL, NT, C], f32, tag="log")
            E4 = rpool.tile([128, BL, NT, C], f16, tag="e4")
            CT = rpool.tile([128, BL, NT, C], f16, tag="ct")
            DEN = rpool.tile([128, BL, NT], f32, tag="den")
            REC = rpool.tile([128, BL, NT], f32, tag="rec")
            VC = rpool.tile([128, BL * 4], f32, tag="vc")
            VBD = rpool.tile([128, BL, 4, C], f16, tag="vbd")

            # ---- build u_hat ----
            with (
                tc.tile_pool(name="bld", bufs=3) as bpool,
                tc.tile_pool(name="bldp", bufs=2, space="PSUM") as bppool,
            ):
                for j in range(128):
                    wt = bpool.tile([128, 512], f16, tag="wt")
                    nc.sync.dma_start(wt[:], wst_d[j])
                    ut = bpool.tile([128, 128], f16, tag="ut")
                    nc.sync.dma_start(ut[:], ubd_d[j])
                    pb = bppool.tile([128, 512], f32, tag="pb")
                    nc.tensor.matmul(pb[:], ut[:], wt[:], start=True, stop=True)
                    st = bpool.tile([128, 512], f16, tag="st")
                    if j % 2 == 0:
                        nc.scalar.copy(st[:], pb[:])
                    else:
                        nc.vector.tensor_copy(st[:], pb[:])
                    # chunk j covers n = 16j + nn -> partitions 16*(j%8)+nn,
                    # ntile j//8; scatter rows (nn,b) of st across 16 partitions
                    nc.sync.dma_start(UA[ds(16 * (j % 8), 16), j // 8, :, :], st[:])

            # ---- routing iterations ----
            with (
                tc.tile_pool(name="it", bufs=2) as ipool,
                tc.tile_pool(name="tb", bufs=8) as tbpool,
                tc.tile_pool(name="ps4", bufs=2, space="PSUM") as s4pool,
                tc.tile_pool(name="pagr", bufs=4, space="PSUM") as agrpool,
                tc.tile_pool(name="dsc", bufs=2, space="DRAM") as dpool,
            ):
                for r in range(R_ITERS):
                    if r > 0:
                        for half in range(2):
                            pas = []
                            for _pi in range(4):
                                pa = agrpool.tile([128, 512], f32, tag="agr")
                                pas.append(pa)
                            for nt in range(NT):
                                # batched xbar transpose: 4 batches x 4 chunks
                                # TB[cl, 4*bi+k, n] = UA[n, nt, b0+bi, 128k+cl]
                                tb = tbpool.tile([128, 16, 128], f16, tag="tb")
                                nc.sync.dma_start_transpose(
                                    tb[:], UA[:, nt, ds(4 * half, 4), :]
                                )
                                for bi in range(4):
                                    for k in range(4):
                                        nc.tensor.matmul(
                                            pas[bi][:, ds(32 * nt, 32)],
                                            tb[:, 4 * bi + k, :],
                                            VBD[:, 4 * half + bi, k, :],
                                            start=(k == 0),
                                            stop=(k == 3),
                                        )
                            for bi in range(4):
                                b = 4 * half + bi
                                lv = LOG[:, b]
                                pav = pas[bi][:].rearrange(
                                    "p (nt c) -> p nt c", c=C
                                )
                                if r == 1:
                                    nc.vector.tensor_copy(lv, pav)
                                else:
                                    nc.vector.tensor_add(lv, lv, pav)
                                nc.scalar.activation(E4[:, b], lv, Exp)
                                nc.vector.reduce_sum(DEN[:, b], E4[:, b], axis=AX)
                                nc.vector.reciprocal(REC[:, b], DEN[:, b])
                                nc.vector.tensor_mul(
                                    CT[:, b],
                                    E4[:, b],
                                    REC[:, b]
                                    .unsqueeze(-1)
                                    .broadcast_to((128, NT, C)),
                                )
                    for g in range(4):
                        ps = s4pool.tile([128, 512], f32, tag="s4")
                        for bi in range(2):
                            b = 2 * g + bi
                            for nt in range(NT):
                                lhsT = C0[:] if r == 0 else CT[:, b, nt, :]
                                nc.tensor.matmul(
                                    ps[ds(64 * bi, 32), :],
                                    lhsT,
                                    UA[:, nt, b, :],
                                    start=(nt == 0),
                                    stop=(nt == NT - 1),
                                )
                        if r < R_ITERS - 1:
                            # dv layout: [cg 8, l 16, bt 2, kk 4] contiguous
                            dv = dpool.tile([128, 8], f32, tag="dv")
                            dvv = dv[:].rearrange(
                                "(cg l) (bt kk) -> cg l bt kk", l=16, kk=4
                            )
                        for bi in range(2):
                            pr = ps[ds(64 * bi, 32), :]
                            mskd = ipool.tile([32, 512], f32, tag="mskd")
                            nc.vector.tensor_mul(mskd[:], pr, MSK[0:32, :])
                            s4r = ipool.tile([32, 16], f32, tag="s4r")
                            nc.vector.reduce_sum(
                                s4r[:],
                                mskd[:].rearrange("p (c l) -> p l c", l=L),
                                axis=AX,
                            )
                            s4b = ipool.tile([32, 16], f32, tag="s4b")
                            nc.vector.tensor_add(s4b[:], s4r[:], BIAS4[0:32, :])
                            sq = ipool.tile([32, 16], f32, tag="sq")
                            n2 = ipool.tile([32, 1], f32, tag="n2")
                            nc.scalar.activation(
                                sq[:], s4b[:], Square, accum_out=n2[:]
                            )
                            n2p = ipool.tile([32, 1], f32, tag="n2p")
                            nc.vector.tensor_scalar_add(n2p[:], n2[:], EPS)
                            tq = ipool.tile([32, 1], f32, tag="tq")
                            nc.scalar.activation(tq[:], n2p[:], Sqrt)
                            m1 = ipool.tile([32, 1], f32, tag="m1")
                            nc.vector.tensor_scalar_add(m1[:], n2p[:], 1.0)
                            dq = ipool.tile([32, 1], f32, tag="dq")
                            nc.vector.tensor_mul(dq[:], m1[:], tq[:])
                            rq = ipool.tile([32, 1], f32, tag="rq")
                            nc.vector.reciprocal(rq[:], dq[:])
                            al = ipool.tile([32, 1], f32, tag="al")
                            nc.vector.tensor_mul(al[:], n2p[:], rq[:])
                            v4 = ipool.tile([32, 16], f32, tag="v4")
                            nc.vector.tensor_scalar_mul(v4[:], s4b[:], al[:])
                            if r < R_ITERS - 1:
                                for kk in range(4):
                                    nc.sync.dma_start(
                                        dvv[:, :, bi, kk], v4[ds(8 * kk, 8), :]
                                    )
                            else:
                                nc.sync.dma_start(
                                    vout_d[ds(64 * g + 32 * bi, 32), :], v4[:]
                                )
                        if r < R_ITERS - 1:
                            nc.gpsimd.dma_start(VC[:, ds(8 * g, 8)], dv[:])
                    if r < R_ITERS - 1:
                        nc.vector.tensor_mul(
                            VBD[:],
                            EALL[:]
                            .rearrange("p (k c) -> p k c", c=C)
                            .unsqueeze(1)
                            .broadcast_to((128, BL, 4, C)),
                            VC[:]
                            .rearrange("p (b k) -> p b k", k=4)
                            .unsqueeze(-1)
                            .broadcast_to((128, BL, 4, C)),
                        )
    nc.compile()
    return nc


def _prep_inputs(x, W, bias):
    """Host-side prep of per-core input maps."""
    u = np.ascontiguousarray(x.reshape(B, N, IL))
    W = np.ascontiguousarray(W)

    wst = W.reshape(128, 128, 512).astype(np.float16)
    c0 = np.full((128, 32), 1.0 / 32.0, np.float16)
    p = np.arange(128)[:, None]
    cl = np.arange(512)[None, :]
    msk = (cl // 16 == p % 32).astype(np.float16)
    kk = np.arange(128)[None, :] // 32
    cp = np.arange(128)[None, :] % 32
    eall = (cp == 8 * kk + p // 16).astype(np.float16)
    bias4 = np.tile(bias.astype(np.float32), (4, 1)).reshape(128, 16)
    bias4 = np.ascontiguousarray(bias4)

    in_maps = []
    for core in range(NCORES):
        ub = u[core * BL : (core + 1) * BL]  # [8, 2048, 8]
        # A[j, nn, i, b] = u[b, 16*j + nn, i]
        A = ub.reshape(BL, 128, 16, IL).transpose(1, 2, 3, 0)
        z = np.zeros((128, 16, IL, 16, BL), np.float16)
        ix = np.arange(16)
        # z[j, nn, i, nn, b] = A[j, nn, i, b]
        z[:, ix, :, ix, :] = A.transpose(1, 0, 2, 3).astype(np.float16)
        ubd = z.reshape(128, 128, 128)
        in_maps.append(
            {
                "ubd": ubd,
                "wst": wst,
                "c0": c0,
                "msk": msk,
                "eall": eall,
                "bias4": bias4,
            }
        )
    return in_maps


def _assemble_output(results):
    out = np.empty((B, C, L), np.float32)
    for core in range(NCORES):
        vout = results[core]["vout"]  # [256, 16]
        v = vout.reshape(2, 4, C, L).reshape(BL, C, L)
        out[core * BL : (core + 1) * BL] = v
    return out


_CACHE = {}


def kernel(x, W, bias):
    from concourse.bass_utils import run_bass_kernel_spmd

    if "nc" not in _CACHE:
        _CACHE["nc"] = _build_program()
    nc = _CACHE["nc"]
    in_maps = _prep_inputs(
        np.asarray(x, np.float32), np.asarray(W, np.float32), np.asarray(bias, np.float32)
    )
    res = run_bass_kernel_spmd(nc, in_maps, core_ids=list(range(NCORES)))
    return _assemble_output(res.results)


# revision 14
# speedup vs baseline: 1.0491x; 1.0491x over previous
"""DigitCaps dynamic-routing kernel for 8 Trainium2 NeuronCores.

Problem (hardcoded shapes): x [64,8,8,32,8] f32, W [2048,8,512] f32,
bias [32,16] f32 -> v [64,32,16] f32.  3 routing iterations.

Strategy: data-parallel over batch B (8 batches per core, W replicated).
Per core:
  - u_hat = einsum('bji,jik->bjk') built once on the tensor engine via
    block-diagonal lhsT packing (16 n's per matmul, K=128=16n*8i,
    M=128=16n*8b), converted to fp16 and kept *resident in SBUF* in
    layout A: UA[p=n%128, nt=n//128, b, cl]  (128 KB/partition).
  - each routing iteration:
      agreement: per (b,nt,cl-chunk) DMA-xbar-transpose a [128n,128cl]
        chunk of UA into [cl,n] and matmul against a block-diagonal
        Vbd[cl, 32] built from v -> psum[n, 32] accumulated over chunks.
      softmax over c on ACT(exp)+DVE.
      s: matmul lhsT=c[n,32] (fp16) rhs=UA[n,512] -> psum[32c', 512(c,l)]
        for 4 batches per PSUM bank; diagonal blocks extracted with a
        0/1 mask + strided reduce; squash on ACT/DVE.
  - v of the last iteration is written out in a [256,16] scratch layout
    and unscrambled on the host.
"""

import sys

import numpy as np

if "/opt/trn_rl_repo" not in sys.path:
    sys.path.insert(0, "/opt/trn_rl_repo")

B, N, IL = 64, 2048, 8
C, L = 32, 16
CL = C * L  # 512
NCORES = 8
BL = B // NCORES  # 8 batches per core
NT = N // 128  # 16 n-tiles
EPS = 1e-7
R_ITERS = 3


def _build_program():
    import concourse.bacc as bacc
    import concourse.bass as bass
    import concourse.mybir as mybir
    import concourse.tile as tile
    from concourse.bass import ds

    f16 = mybir.dt.float16
    f32 = mybir.dt.float32
    AX = mybir.AxisListType.X
    Exp = mybir.ActivationFunctionType.Exp
    Sqrt = mybir.ActivationFunctionType.Sqrt
    Square = mybir.ActivationFunctionType.Square

    nc = bacc.Bacc()

    ubd_d = nc.dram_tensor("ubd", [128, 128, 128], f16, kind="ExternalInput")
    wst_d = nc.dram_tensor("wst", [128, 128, 512], f16, kind="ExternalInput")
    c0_d = nc.dram_tensor("c0", [128, 32], f16, kind="ExternalInput")
    msk_d = nc.dram_tensor("msk", [128, 512], f16, kind="ExternalInput")
    eall_d = nc.dram_tensor("eall", [128, 128], f16, kind="ExternalInput")
    bias4_d = nc.dram_tensor("bias4", [128, 16], f32, kind="ExternalInput")
    vout_d = nc.dram_tensor("vout", [256, 16], f32, kind="ExternalOutput")

    with tile.TileContext(nc) as tc:
        with tc.tile_pool(name="res", bufs=1) as rpool:
            C0 = rpool.tile([128, 32], f16, tag="c0")
            nc.sync.dma_start(C0[:], c0_d[:, :])
            MSK = rpool.tile([128, 512], f16, tag="msk")
            nc.sync.dma_start(MSK[:], msk_d[:, :])
            EALL = rpool.tile([128, 128], f16, tag="eall")
            nc.sync.dma_start(EALL[:], eall_d[:, :])
            BIAS4 = rpool.tile([128, 16], f32, tag="bias4")
            nc.sync.dma_start(BIAS4[:], bias4_d[:, :])

            UA = rpool.tile([128, NT, BL, CL], f16, tag="ua")
            LOG = rpool.tile([128, BL, NT, C], f32, tag="log")
            E4 = rpool.tile([128, BL, NT, C], f16, tag="e4")
            CT = rpool.tile([128, BL, NT, C], f16, tag="ct")
            DEN = rpool.tile([128, BL, NT], f32, tag="den")
            REC = rpool.tile([128, BL, NT], f32, tag="rec")
            VC = rpool.tile([128, BL * 4], f32, tag="vc")
            VBD = rpool.tile([128, BL, 4, C], f16, tag="vbd")

            # ---- build u_hat ----
            with (
                tc.tile_pool(name="bld", bufs=4) as bpool,
                tc.tile_pool(name="bldp", bufs=3, space="PSUM") as bppool,
            ):
                for j in range(128):
                    eng_a = nc.sync if j % 2 == 0 else nc.scalar
                    eng_b = nc.scalar if j % 2 == 0 else nc.sync
                    wt = bpool.tile([128, 512], f16, tag="wt")
                    eng_a.dma_start(wt[:], wst_d[j])
                    ut = bpool.tile([128, 128], f16, tag="ut")
                    eng_b.dma_start(ut[:], ubd_d[j])
                    pb = bppool.tile([128, 512], f32, tag="pb")
                    nc.tensor.matmul(pb[:], ut[:], wt[:], start=True, stop=True)
                    st = bpool.tile([128, 512], f16, tag="st")
                    nc.vector.tensor_copy(st[:], pb[:])
                    # chunk j covers n = 16j + nn -> partitions 16*(j%8)+nn,
                    # ntile j//8; scatter rows (nn,b) of st across 16 partitions
                    eng_b.dma_start(UA[ds(16 * (j % 8), 16), j // 8, :, :], st[:])

            # ---- routing iterations ----
            with (
                tc.tile_pool(name="it", bufs=2) as ipool,
                tc.tile_pool(name="tb", bufs=8) as tbpool,
                tc.tile_pool(name="ps4", bufs=2, space="PSUM") as s4pool,
                tc.tile_pool(name="pagr", bufs=4, space="PSUM") as agrpool,
                tc.tile_pool(name="dsc", bufs=2, space="DRAM") as dpool,
            ):
                for r in range(R_ITERS):
                    if r > 0:
                        for half in range(2):
                            pas = []
                            for _pi in range(4):
                                pa = agrpool.tile([128, 512], f32, tag="agr")
                                pas.append(pa)
                            for nt in range(NT):
                                # batched xbar transpose: 4 batches x 4 chunks
                                # TB[cl, 4*bi+k, n] = UA[n, nt, b0+bi, 128k+cl]
                                tb = tbpool.tile([128, 16, 128], f16, tag="tb")
                                nc.sync.dma_start_transpose(
                                    tb[:], UA[:, nt, ds(4 * half, 4), :]
                                )
                                for bi in range(4):
                                    for k in range(4):
                                        nc.tensor.matmul(
                                            pas[bi][:, ds(32 * nt, 32)],
                                            tb[:, 4 * bi + k, :],
                                            VBD[:, 4 * half + bi, k, :],
                                            start=(k == 0),
                                            stop=(k == 3),
                                        )
                            for bi in range(4):
                                b = 4 * half + bi
                                lv = LOG[:, b]
                                pav = pas[bi][:].rearrange(
                                    "p (nt c) -> p nt c", c=C
                                )
                                if r == 1:
                                    nc.vector.tensor_copy(lv, pav)
                                else:
                                    nc.vector.tensor_add(lv, lv, pav)
                                nc.scalar.activation(E4[:, b], lv, Exp)
                                nc.vector.reduce_sum(DEN[:, b], E4[:, b], axis=AX)
                                nc.vector.reciprocal(REC[:, b], DEN[:, b])
                                nc.vector.tensor_mul(
                                    CT[:, b],
                                    E4[:, b],
                                    REC[:, b]
                                    .unsqueeze(-1)
                                    .broadcast_to((128, NT, C)),
                                )
                    for g in range(4):
                        ps = s4pool.tile([128, 512], f32, tag="s4")
                        for bi in range(2):
                            b = 2 * g + bi
                            for nt in range(NT):
                                lhsT = C0[:] if r == 0 else CT[:, b, nt, :]
                                nc.tensor.matmul(
                                    ps[ds(64 * bi, 32), :],
                                    lhsT,
                                    UA[:, nt, b, :],
                                    start=(nt == 0),
                                    stop=(nt == NT - 1),
                                )
                        if r < R_ITERS - 1:
                            # dv layout: [cg 8, l 16, bt 2, kk 4] contiguous
                            dv = dpool.tile([128, 8], f32, tag="dv")
                            dvv = dv[:].rearrange(
                                "(cg l) (bt kk) -> cg l bt kk", l=16, kk=4
                            )
                        for bi in range(2):
                            pr = ps[ds(64 * bi, 32), :]
                            mskd = ipool.tile([32, 512], f32, tag="mskd")
                            nc.vector.tensor_mul(mskd[:], pr, MSK[0:32, :])
                            s4r = ipool.tile([32, 16], f32, tag="s4r")
                            nc.vector.reduce_sum(
                                s4r[:],
                                mskd[:].rearrange("p (c l) -> p l c", l=L),
                                axis=AX,
                            )
                            s4b = ipool.tile([32, 16], f32, tag="s4b")
                            nc.vector.tensor_add(s4b[:], s4r[:], BIAS4[0:32, :])
                            sq = ipool.tile([32, 16], f32, tag="sq")
                            n2 = ipool.tile([32, 1], f32, tag="n2")
                            nc.scalar.activation(
                                sq[:], s4b[:], Square, accum_out=n2[:]
                            )
                            n2p = ipool.tile([32, 1], f32, tag="n2p")
                            nc.vector.tensor_scalar_add(n2p[:], n2[:], EPS)
                            tq = ipool.tile([32, 1], f32, tag="tq")
                            nc.scalar.activation(tq[:], n2p[:], Sqrt)
                            m1 = ipool.tile([32, 1], f32, tag="m1")
                            nc.vector.tensor_scalar_add(m1[:], n2p[:], 1.0)
                            dq = ipool.tile([32, 1], f32, tag="dq")
                            nc.vector.tensor_mul(dq[:], m1[:], tq[:])
                            rq = ipool.tile([32, 1], f32, tag="rq")
                            nc.vector.reciprocal(rq[:], dq[:])
                            al = ipool.tile([32, 1], f32, tag="al")
                            nc.vector.tensor_mul(al[:], n2p[:], rq[:])
                            v4 = ipool.tile([32, 16], f32, tag="v4")
                            nc.vector.tensor_scalar_mul(v4[:], s4b[:], al[:])
                            if r < R_ITERS - 1:
                                for kk in range(4):
                                    nc.sync.dma_start(
                                        dvv[:, :, bi, kk], v4[ds(8 * kk, 8), :]
                                    )
                            else:
                                nc.sync.dma_start(
                                    vout_d[ds(64 * g + 32 * bi, 32), :], v4[:]
                                )
                        if r < R_ITERS - 1:
                            nc.gpsimd.dma_start(VC[:, ds(8 * g, 8)], dv[:])
                    if r < R_ITERS - 1:
                        nc.vector.tensor_mul(
                            VBD[:],
                            EALL[:]
                            .rearrange("p (k c) -> p k c", c=C)
                            .unsqueeze(1)
                            .broadcast_to((128, BL, 4, C)),
                            VC[:]
                            .rearrange("p (b k) -> p b k", k=4)
                            .unsqueeze(-1)
                            .broadcast_to((128, BL, 4, C)),
                        )
    nc.compile()
    return nc


def _prep_inputs(x, W, bias):
    """Host-side prep of per-core input maps."""
    u = np.ascontiguousarray(x.reshape(B, N, IL))
    W = np.ascontiguousarray(W)

    wst = W.reshape(128, 128, 512).astype(np.float16)
    c0 = np.full((128, 32), 1.0 / 32.0, np.float16)
    p = np.arange(128)[:, None]
    cl = np.arange(512)[None, :]
    msk = (cl // 16 == p % 32).astype(np.float16)
    kk = np.arange(128)[None, :] // 32
    cp = np.arange(128)[None, :] % 32
    eall = (cp == 8 * kk + p // 16).astype(np.float16)
    bias4 = np.tile(bias.astype(np.float32), (4, 1)).reshape(128, 16)
    bias4 = np.ascontiguousarray(bias4)

    in_maps = []
    for core in range(NCORES):
        ub = u[core * BL : (core + 1) * BL]  # [8, 2048, 8]
        # A[j, nn, i, b] = u[b, 16*j + nn, i]
        A = ub.reshape(BL, 128, 16, IL).transpose(1, 2, 3, 0)
        z = np.zeros((128, 16, IL, 16, BL), np.float16)
        ix = np.arange(16)
        # z[j, nn, i, nn, b] = A[j, nn, i, b]
        z[:, ix, :, ix, :] = A.transpose(1, 0, 2, 3).astype(np.float16)
        ubd = z.reshape(128, 128, 128)
        in_maps.append(
            {
                "ubd": ubd,
                "wst": wst,
                "c0": c0,
                "msk": msk,
                "eall": eall,
                "bias4": bias4,
            }
        )
    return in_maps


def _assemble_output(results):
    out = np.empty((B, C, L), np.float32)
    for core in range(NCORES):
        vout = results[core]["vout"]  # [256, 16]
        v = vout.reshape(2, 4, C, L).reshape(BL, C, L)
        out[core * BL : (core + 1) * BL] = v
    return out


_CACHE = {}


def kernel(x, W, bias):
    from concourse.bass_utils import run_bass_kernel_spmd

    if "nc" not in _CACHE:
        _CACHE["nc"] = _build_program()
    nc = _CACHE["nc"]
    in_maps = _prep_inputs(
        np.asarray(x, np.float32), np.asarray(W, np.float32), np.asarray(bias, np.float32)
    )
    res = run_bass_kernel_spmd(nc, in_maps, core_ids=list(range(NCORES)))
    return _assemble_output(res.results)


# revision 16
# speedup vs baseline: 1.2086x; 1.1520x over previous
"""DigitCaps dynamic-routing kernel for 8 Trainium2 NeuronCores.

Problem (hardcoded shapes): x [64,8,8,32,8] f32, W [2048,8,512] f32,
bias [32,16] f32 -> v [64,32,16] f32.  3 routing iterations.

Strategy: data-parallel over batch B (8 batches per core, W replicated).
Per core:
  - u_hat = einsum('bji,jik->bjk') built once on the tensor engine via
    block-diagonal lhsT packing (16 n's per matmul, K=128=16n*8i,
    M=128=16n*8b), converted to fp16 and kept *resident in SBUF* in
    layout A: UA[p=n%128, nt=n//128, b, cl]  (128 KB/partition).
  - each routing iteration:
      agreement: per (b,nt,cl-chunk) DMA-xbar-transpose a [128n,128cl]
        chunk of UA into [cl,n] and matmul against a block-diagonal
        Vbd[cl, 32] built from v -> psum[n, 32] accumulated over chunks.
      softmax over c on ACT(exp)+DVE.
      s: matmul lhsT=c[n,32] (fp16) rhs=UA[n,512] -> psum[32c', 512(c,l)]
        for 4 batches per PSUM bank; diagonal blocks extracted with a
        0/1 mask + strided reduce; squash on ACT/DVE.
  - v of the last iteration is written out in a [256,16] scratch layout
    and unscrambled on the host.
"""

import sys

import numpy as np

if "/opt/trn_rl_repo" not in sys.path:
    sys.path.insert(0, "/opt/trn_rl_repo")

B, N, IL = 64, 2048, 8
C, L = 32, 16
CL = C * L  # 512
NCORES = 8
BL = B // NCORES  # 8 batches per core
NT = N // 128  # 16 n-tiles
EPS = 1e-7
R_ITERS = 3


def _build_program():
    import concourse.bacc as bacc
    import concourse.bass as bass
    import concourse.mybir as mybir
    import concourse.tile as tile
    from concourse.bass import ds

    f16 = mybir.dt.float16
    f32 = mybir.dt.float32
    AX = mybir.AxisListType.X
    Exp = mybir.ActivationFunctionType.Exp
    Sqrt = mybir.ActivationFunctionType.Sqrt
    Square = mybir.ActivationFunctionType.Square

    nc = bacc.Bacc()

    ubd_d = nc.dram_tensor("ubd", [128, 128, 128], f16, kind="ExternalInput")
    wst_d = nc.dram_tensor("wst", [128, 128, 512], f16, kind="ExternalInput")
    c0_d = nc.dram_tensor("c0", [128, 32], f16, kind="ExternalInput")
    msk_d = nc.dram_tensor("msk", [128, 512], f16, kind="ExternalInput")
    eall_d = nc.dram_tensor("eall", [128, 128], f16, kind="ExternalInput")
    bias4_d = nc.dram_tensor("bias4", [128, 16], f32, kind="ExternalInput")
    vout_d = nc.dram_tensor("vout", [256, 16], f32, kind="ExternalOutput")

    with tile.TileContext(nc) as tc:
        with tc.tile_pool(name="res", bufs=1) as rpool:
            C0 = rpool.tile([128, 32], f16, tag="c0")
            nc.sync.dma_start(C0[:], c0_d[:, :])
            MSK = rpool.tile([128, 512], f16, tag="msk")
            nc.sync.dma_start(MSK[:], msk_d[:, :])
            EALL = rpool.tile([128, 128], f16, tag="eall")
            nc.sync.dma_start(EALL[:], eall_d[:, :])
            BIAS4 = rpool.tile([128, 16], f32, tag="bias4")
            nc.sync.dma_start(BIAS4[:], bias4_d[:, :])

            UA = rpool.tile([128, NT, BL, CL], f16, tag="ua")
            LOG = rpool.tile([128, BL, NT, C], f32, tag="log")
            E4 = rpool.tile([128, BL, NT, C], f16, tag="e4")
            CT = rpool.tile([128, BL, NT, C], f16, tag="ct")
            DEN = rpool.tile([128, BL, NT], f32, tag="den")
            REC = rpool.tile([128, BL, NT], f32, tag="rec")
            VC = rpool.tile([128, BL * 4], f32, tag="vc")
            VBD = rpool.tile([128, BL, 4, C], f16, tag="vbd")

            # ---- build u_hat ----
            with (
                tc.tile_pool(name="bld", bufs=4) as bpool,
                tc.tile_pool(name="bldp", bufs=3, space="PSUM") as bppool,
            ):
                for j in range(128):
                    eng_a = nc.sync if j % 2 == 0 else nc.scalar
                    eng_b = nc.scalar if j % 2 == 0 else nc.sync
                    wt = bpool.tile([128, 512], f16, tag="wt")
                    eng_a.dma_start(wt[:], wst_d[j])
                    ut = bpool.tile([128, 128], f16, tag="ut")
                    eng_b.dma_start(ut[:], ubd_d[j])
                    pb = bppool.tile([128, 512], f32, tag="pb")
                    nc.tensor.matmul(pb[:], ut[:], wt[:], start=True, stop=True)
                    st = bpool.tile([128, 512], f16, tag="st")
                    nc.vector.tensor_copy(st[:], pb[:])
                    # chunk j covers n = 16j + nn -> partitions 16*(j%8)+nn,
                    # ntile j//8; scatter rows (nn,b) of st across 16 partitions
                    eng_b.dma_start(UA[ds(16 * (j % 8), 16), j // 8, :, :], st[:])

            # ---- routing iterations ----
            with (
                tc.tile_pool(name="it", bufs=2) as ipool,
                tc.tile_pool(name="tb", bufs=8) as tbpool,
                tc.tile_pool(name="ps4", bufs=2, space="PSUM") as s4pool,
                tc.tile_pool(name="pagr", bufs=4, space="PSUM") as agrpool,
                tc.tile_pool(name="dsc", bufs=2, space="DRAM") as dpool,
            ):
                for r in range(R_ITERS):
                    if r > 0:
                        for half in range(2):
                            pas = []
                            for _pi in range(4):
                                pa = agrpool.tile([128, 512], f32, tag="agr")
                                pas.append(pa)
                            for nt in range(NT):
                                # batched xbar transpose: 4 batches x 4 chunks
                                # TB[cl, 4*bi+k, n] = UA[n, nt, b0+bi, 128k+cl]
                                tb = tbpool.tile([128, 16, 128], f16, tag="tb")
                                nc.sync.dma_start_transpose(
                                    tb[:], UA[:, nt, ds(4 * half, 4), :]
                                )
                                for bi in range(4):
                                    for k in range(4):
                                        nc.tensor.matmul(
                                            pas[bi][:, ds(32 * nt, 32)],
                                            tb[:, 4 * bi + k, :],
                                            VBD[:, 4 * half + bi, k, :],
                                            start=(k == 0),
                                            stop=(k == 3),
                                        )
                            for bi in range(4):
                                b = 4 * half + bi
                                lv = LOG[:, b]
                                pav = pas[bi][:].rearrange(
                                    "p (nt c) -> p nt c", c=C
                                )
                                if r == 1:
                                    nc.vector.tensor_copy(lv, pav)
                                else:
                                    nc.vector.tensor_add(lv, lv, pav)
                                nc.scalar.activation(E4[:, b], lv, Exp)
                                nc.vector.reduce_sum(DEN[:, b], E4[:, b], axis=AX)
                                nc.vector.reciprocal(REC[:, b], DEN[:, b])
                                nc.vector.tensor_mul(
                                    CT[:, b],
                                    E4[:, b],
                                    REC[:, b]
                                    .unsqueeze(-1)
                                    .broadcast_to((128, NT, C)),
                                )
                    for g in range(4):
                        ps = s4pool.tile([128, 512], f32, tag="s4")
                        for bi in range(2):
                            b = 2 * g + bi
                            for nt in range(NT):
                                lhsT = C0[:] if r == 0 else CT[:, b, nt, :]
                                nc.tensor.matmul(
                                    ps[ds(64 * bi, 32), :],
                                    lhsT,
                                    UA[:, nt, b, :],
                                    start=(nt == 0),
                                    stop=(nt == NT - 1),
                                )
                        if r < R_ITERS - 1:
                            # dv layout: [cg 8, l 16, bt 2, kk 4] contiguous
                            dv = dpool.tile([128, 8], f32, tag="dv")
                            dvv = dv[:].rearrange(
                                "(cg l) (bt kk) -> cg l bt kk", l=16, kk=4
                            )
                        for bi in range(2):
                            pr = ps[ds(64 * bi, 32), :]
                            mskd = ipool.tile([32, 512], f32, tag="mskd")
                            nc.vector.tensor_mul(mskd[:], pr, MSK[0:32, :])
                            s4r = ipool.tile([32, 16], f32, tag="s4r")
                            nc.vector.reduce_sum(
                                s4r[:],
                                mskd[:].rearrange("p (c l) -> p l c", l=L),
                                axis=AX,
                            )
                            s4b = ipool.tile([32, 16], f32, tag="s4b")
                            nc.vector.tensor_add(s4b[:], s4r[:], BIAS4[0:32, :])
                            sq = ipool.tile([32, 16], f32, tag="sq")
                            n2 = ipool.tile([32, 1], f32, tag="n2")
                            nc.scalar.activation(
                                sq[:], s4b[:], Square, accum_out=n2[:]
                            )
                            n2p = ipool.tile([32, 1], f32, tag="n2p")
                            nc.vector.tensor_scalar_add(n2p[:], n2[:], EPS)
                            tq = ipool.tile([32, 1], f32, tag="tq")
                            nc.scalar.activation(tq[:], n2p[:], Sqrt)
                            m1 = ipool.tile([32, 1], f32, tag="m1")
                            nc.vector.tensor_scalar_add(m1[:], n2p[:], 1.0)
                            dq = ipool.tile([32, 1], f32, tag="dq")
                            nc.vector.tensor_mul(dq[:], m1[:], tq[:])
                            rq = ipool.tile([32, 1], f32, tag="rq")
                            nc.vector.reciprocal(rq[:], dq[:])
                            al = ipool.tile([32, 1], f32, tag="al")
                            nc.vector.tensor_mul(al[:], n2p[:], rq[:])
                            v4 = ipool.tile([32, 16], f32, tag="v4")
                            nc.vector.tensor_scalar_mul(v4[:], s4b[:], al[:])
                            if r < R_ITERS - 1:
                                for kk in range(4):
                                    nc.sync.dma_start(
                                        dvv[:, :, bi, kk], v4[ds(8 * kk, 8), :]
                                    )
                            else:
                                nc.sync.dma_start(
                                    vout_d[ds(64 * g + 32 * bi, 32), :], v4[:]
                                )
                        if r < R_ITERS - 1:
                            nc.gpsimd.dma_start(VC[:, ds(8 * g, 8)], dv[:])
                    if r < R_ITERS - 1:
                        nc.vector.tensor_mul(
                            VBD[:],
                            EALL[:]
                            .rearrange("p (k c) -> p k c", c=C)
                            .unsqueeze(1)
                            .broadcast_to((128, BL, 4, C)),
                            VC[:]
                            .rearrange("p (b k) -> p b k", k=4)
                            .unsqueeze(-1)
                            .broadcast_to((128, BL, 4, C)),
                        )
    nc.compile()
    return nc


def _prep_inputs(x, W, bias):
    """Host-side prep of per-core input maps."""
    u = np.ascontiguousarray(x.reshape(B, N, IL))
    W = np.ascontiguousarray(W)

    wst = W.reshape(128, 128, 512).astype(np.float16)
    c0 = np.full((128, 32), 1.0 / 32.0, np.float16)
    p = np.arange(128)[:, None]
    cl = np.arange(512)[None, :]
    msk = (cl // 16 == p % 32).astype(np.float16)
    kk = np.arange(128)[None, :] // 32
    cp = np.arange(128)[None, :] % 32
    eall = (cp == 8 * kk + p // 16).astype(np.float16)
    bias4 = np.tile(bias.astype(np.float32), (4, 1)).reshape(128, 16)
    bias4 = np.ascontiguousarray(bias4)

    in_maps = []
    for core in range(NCORES):
        ub = u[core * BL : (core + 1) * BL]  # [8, 2048, 8]
        # A[j, nn, i, b] = u[b, 16*j + nn, i]
        A = ub.reshape(BL, 128, 16, IL).transpose(1, 2, 3, 0)
        z = np.zeros((128, 16, IL, 16, BL), np.float16)
        ix = np.arange(16)
        # z[j, nn, i, nn, b] = A[j, nn, i, b]
        z[:, ix, :, ix, :] = A.transpose(1, 0, 2, 3).astype(np.float16)
        ubd = z.reshape(128, 128, 128)
        in_maps.append(
            {
                "ubd": ubd,
                "wst": wst,
                "c0": c0,
                "msk": msk,
                "eall": eall,
                "bias4": bias4,
            }
        )
    return in_maps


def _assemble_output(results):
    out = np.empty((B, C, L), np.float32)
    for core in range(NCORES):
        vout = results[core]["vout"]  # [256, 16]
        v = vout.reshape(2, 4, C, L).reshape(BL, C, L)
        out[core * BL : (core + 1) * BL] = v
    return out


_CACHE = {}


def kernel(x, W, bias):
    from concourse.bass_utils import run_bass_kernel_spmd

    if "nc" not in _CACHE:
        _CACHE["nc"] = _build_program()
    nc = _CACHE["nc"]
    in_maps = _prep_inputs(
        np.asarray(x, np.float32), np.asarray(W, np.float32), np.asarray(bias, np.float32)
    )
    res = run_bass_kernel_spmd(nc, in_maps, core_ids=list(range(NCORES)))
    return _assemble_output(res.results)
